# revision 8
# baseline (speedup 1.0000x reference)
"""Trainium2 Bass kernel for nn_NeuralCellularAutomata2 (B16,H64,W64,C256).

Self-contained: hardcodes shapes/sharding. Strategy:
 - data-parallel over batch: 16 images -> 8 cores x 2 images
 - the axon tunnel (host<->device transfer) is the bottleneck, so the wire
   format is aggressively compressed and all layout work is done on device:
     h ships as fp8 e3m4 in raw NHWC order (one contiguous cast on host,
     no transpose/pad); the device transposes to channel-major via PE
     identity matmuls and zero-pads in SBUF.
     conv+up1 weights ship UNFOLDED (W1 in fp8 e4m3 x16 + 3x3 taps as f32
     per-partition scalars) and are folded on device by short DVE chains
     into 9 fused [2C,C] bf16 matrices => 9 shifted PSUM-accumulated
     matmuls; qkv is folded host-side into A = Wq^T Wk / sqrt(C) (x256 in
     fp8 e4m3, descaled by the PSUM-copy activation's scale) so
     scores = h . (A h)_shifted and q,k are never built.
     remaining weights pack into two small arrays (bf16: w2t|wvt,
     fp8: at|i256) plus one f32 scalar array (biases|mask|taps).
     the device returns delta = out - h in fp8 e3m4 (small magnitude);
     the host reconstructs out = h_f32 + delta in full precision.
 - device per core:
     transpose/pad -> ST1 fused conv+up1 -> GELU -> up2 -> residual h_new
     z = A h_new; Gram G = h_new^T z over 4-row bands; 9 score diagonals
     extracted via DRAM roundtrip with stride-259 access patterns;
     softmax in pixel-partition layout; weighted v-sum as PE matmul
     against a banded W' matrix built by diagonal DMA scatter to DRAM;
     h_new^T via identity matmul in the same PSUM tile; delta = psum - x
     fused in the PSUM->SBUF eviction.
"""
import math

import numpy as np
import ml_dtypes

import concourse.bass as bass
import concourse.tile as tile
from concourse import bacc, mybir
from concourse.bass_utils import run_bass_kernel_spmd

B, H, W, C = 16, 64, 64, 256
NCORES = 8
BS = B // NCORES          # images per core
C2, C3 = 2 * C, 3 * C
HW = H * W                # 4096 pixels per image
NT = 8                    # 512-pixel tiles per image
NCHUNK = HW // 128        # 32 x 128-pixel chunks per image
ZP = 1 + 66 * 64 + 1      # padded-z flat length (guard + 66 rows + guard)
W1SC = 16.0               # fp8 shipping scale for W1 (descaled via taps)
W2SC = 1.0                # w2t ships bf16, no scale
ASC = 256.0               # fp8 shipping scale for A (descaled in z copy)

F32 = mybir.dt.float32
BF16 = mybir.dt.bfloat16
F8E3 = mybir.dt.float8e3   # e3m4: max 15.5, best for ~N(0,1) data
F8E4 = mybir.dt.float8e4   # e4m3

NP_E3 = ml_dtypes.float8_e3m4
NP_E4 = ml_dtypes.float8_e4m3
NP_BF = ml_dtypes.bfloat16

_TAUS = [(dy, dx) for dy in (-1, 0, 1) for dx in (-1, 0, 1)]


def _cap(ap, offset, dims):
    """Build a custom access pattern on ap's tensor: dims = [(step, count)...]."""
    a = ap.copy()
    a.offset = offset
    v = a.ap
    v.clear()
    v.extend([(int(s), int(n)) for (s, n) in dims])
    return a


def _build_program(reps=1):
    nc = bacc.Bacc(
        trn_type="TRN2", target_bir_lowering=False, debug=False,
        num_devices=NCORES,
    )
    # ---- DRAM I/O (per-core), wire-compressed dtypes, few arrays.
    hraw_d = nc.dram_tensor("hraw", [BS, 32, 128, 256], F8E3,
                            kind="ExternalInput").ap()
    w1t_d = nc.dram_tensor("w1t", [3, 2, 4, 128, 128], F8E4,
                           kind="ExternalInput").ap()
    # wpk8: [at x256 (4x128) | i256 (2x256)] in e4m3
    wpk8_d = nc.dram_tensor("wpk8", [128, 1024], F8E4,
                            kind="ExternalInput").ap()
    # wpkb: [w2t (8x128) | wvt (2x256)] in bf16
    wpkb_d = nc.dram_tensor("wpkb", [128, 1536], BF16,
                            kind="ExternalInput").ap()
    # bk: [bh (4) | b2 (2) | mask (9) | taps/16 (54)] in f32
    bk_d = nc.dram_tensor("bk", [128, 69], F32, kind="ExternalInput").ap()
    out_d = nc.dram_tensor("out", [BS, 64, 64, 256], F8E3,
                           kind="ExternalOutput").ap()

    GELU = mybir.ActivationFunctionType.Gelu
    EXP = mybir.ActivationFunctionType.Exp
    COPY = mybir.ActivationFunctionType.Copy
    ADD = mybir.AluOpType.add
    MULT = mybir.AluOpType.mult
    SUB = mybir.AluOpType.subtract

    with tile.TileContext(nc) as tc:
        with (
            tc.tile_pool(name="wts", bufs=1) as wts,
            tc.tile_pool(name="konst", bufs=1) as konst,
            tc.tile_pool(name="fsc", bufs=2) as fsc,
            tc.tile_pool(name="data", bufs=2) as data,
            tc.tile_pool(name="hnewp", bufs=2) as hnewp,
            tc.tile_pool(name="zpadp", bufs=2) as zpadp,
            tc.tile_pool(name="hidp", bufs=8) as hidp,
            tc.tile_pool(name="vp", bufs=6) as vpool,
            tc.tile_pool(name="small", bufs=4) as small,
            tc.tile_pool(name="wlp", bufs=6) as wlp,
            tc.tile_pool(name="ps1", bufs=2, space="PSUM") as ps1,
            tc.tile_pool(name="ps2", bufs=1, space="PSUM") as ps2,
            tc.tile_pool(name="ps3", bufs=2, space="PSUM") as ps3,
            tc.tile_pool(name="gdram", bufs=4, space="DRAM") as gdram,
            tc.tile_pool(name="wpdram", bufs=4, space="DRAM") as wpdram,
        ):
            # ---------- load packed weights / constants ----------
            w1t = {}
            for t in range(3):
                for cc in range(2):
                    for mc in range(4):
                        tt = wts.tile([128, 128], F8E4,
                                      name=f"w1t_{t}_{cc}_{mc}")
                        nc.sync.dma_start(tt[:], w1t_d[t, cc, mc])
                        w1t[t, cc, mc] = tt
            wpk8 = wts.tile([128, 1024], F8E4, name="wpk8")
            nc.sync.dma_start(wpk8[:], wpk8_d[:])
            wpkb = wts.tile([128, 1536], BF16, name="wpkb")
            nc.sync.dma_start(wpkb[:], wpkb_d[:])
            bk = konst.tile([128, 69], F32, name="bk")
            nc.sync.dma_start(bk[:], bk_d[:])

            def at_ap(kc, mc):
                c0 = (kc * 2 + mc) * 128
                return wpk8[:, c0:c0 + 128]

            def i256_ap(kc):
                return wpk8[:, 512 + kc * 256:512 + kc * 256 + 256]

            i128_ap = wpk8[:, 512:512 + 128]

            def w2t_ap(kc, mc):
                c0 = (kc * 2 + mc) * 128
                return wpkb[:, c0:c0 + 128]

            def wvt_ap(kc):
                return wpkb[:, 1024 + kc * 256:1024 + kc * 256 + 256]

            def bh_ap(mc):
                return bk[:, mc:mc + 1]

            def b2_ap(mc):
                return bk[:, 4 + mc:5 + mc]

            mask_ap = bk[:, 6:15]

            def wps_ap(cc, t, tau):
                c0 = 15 + cc * 27 + t * 9 + tau
                return bk[:, c0:c0 + 1]

            vzero = konst.tile([128, 256], BF16, name="vzero")
            nc.gpsimd.memset(vzero[:], 0.0)
            wpz = konst.tile([128, 384], BF16, name="wpz")
            nc.gpsimd.memset(wpz[:], 0.0)

            # ---------- device-side fold: w1f[tau] = sum_t w1t[t] * taps ----
            w1f = {}
            for tau in range(9):
                for cc in range(2):
                    for mc in range(4):
                        a0 = fsc.tile([128, 128], F32, name="facc0",
                                      tag="facc0")
                        nc.vector.tensor_scalar(
                            out=a0[:], in0=w1t[0, cc, mc][:],
                            scalar1=wps_ap(cc, 0, tau), scalar2=None,
                            op0=MULT)
                        a1 = fsc.tile([128, 128], F32, name="facc1",
                                      tag="facc1")
                        nc.vector.scalar_tensor_tensor(
                            out=a1[:], in0=w1t[1, cc, mc][:],
                            scalar=wps_ap(cc, 1, tau), in1=a0[:],
                            op0=MULT, op1=ADD)
                        dst = wts.tile([128, 128], BF16,
                                       name=f"w1f_{tau}_{cc}_{mc}")
                        nc.vector.scalar_tensor_tensor(
                            out=dst[:], in0=w1t[2, cc, mc][:],
                            scalar=wps_ap(cc, 2, tau), in1=a1[:],
                            op0=MULT, op1=ADD)
                        w1f[tau, cc, mc] = dst

            # ---------- per-image pipeline ----------
            def run_image(img):
                # NHWC chunks [pix128, chunk32, ch256]
                hh = data.tile([128, 32, 256], F8E3, name="hh", tag="hh")
                nc.sync.dma_start(
                    hh[:], _cap(hraw_d, img * HW * 256,
                                [(256, 128), (32768, 32), (1, 256)]))
                # transpose+pad to channel-major [128ch, 66, 66]
                xr = []
                for cc in range(2):
                    t = data.tile([128, 66, 66], F8E3, name="xr", tag="xr")
                    nc.gpsimd.memset(t[:], 0.0)
                    xr.append(t)
                for p in range(32):
                    tp = ps2.tile([128, 256], F32, space="PSUM",
                                  name="fin_ps", tag="fin_ps")
                    for cc in range(2):
                        nc.tensor.matmul(
                            tp[:, 128 * cc:128 * cc + 128],
                            hh[:, p, 128 * cc:128 * cc + 128],
                            i128_ap, start=True, stop=True)
                    for cc in range(2):
                        for r in range(2):
                            nc.vector.tensor_copy(
                                xr[cc][:, 1 + 2 * p + r, 1:65],
                                tp[:, 128 * cc + 64 * r:
                                      128 * cc + 64 * r + 64])

                h_new = []
                for cc in range(2):
                    h_new.append(hnewp.tile([128, HW], BF16, name="h_new",
                                            tag="h_new"))

                # ---- ST1 fused conv+up1 -> GELU -> up2 -> residual
                for nt in range(NT):
                    r0 = 8 * nt
                    hid_sb = []
                    for mc in range(4):
                        hp = ps1.tile([128, 512], F32, space="PSUM",
                                      name="hid_ps", tag="hid_ps")
                        k = 0
                        for tau, (dy, dx) in enumerate(_TAUS):
                            for cc in range(2):
                                rhs = xr[cc][:, 1 + dy + r0:9 + dy + r0,
                                             1 + dx:65 + dx]
                                nc.tensor.matmul(
                                    hp[:], w1f[tau, cc, mc][:], rhs,
                                    start=(k == 0), stop=(k == 17))
                                k += 1
                        hs = hidp.tile([128, 512], BF16, name="hid_sb",
                                       tag="hid_sb")
                        nc.scalar.activation(hs[:], hp[:], GELU,
                                             bias=bh_ap(mc))
                        hid_sb.append(hs)
                    for mc in range(2):
                        dp = ps2.tile([128, 512], F32, space="PSUM",
                                      name="dx_ps", tag="dx_ps")
                        for kc in range(4):
                            nc.tensor.matmul(dp[:], w2t_ap(kc, mc),
                                             hid_sb[kc][:],
                                             start=(kc == 0), stop=(kc == 3))
                        # h_new = (dx + b2) + x
                        nc.vector.scalar_tensor_tensor(
                            out=h_new[mc][:, 512 * nt:512 * nt + 512],
                            in0=dp[:], scalar=b2_ap(mc),
                            in1=xr[mc][:, 1 + r0:9 + r0, 1:65],
                            op0=ADD, op1=ADD)

                # ---- z = A @ h_new into padded flat layout (descale 1/ASC)
                z_pad = []
                for cc in range(2):
                    zt = zpadp.tile([128, ZP], BF16, name="z_pad",
                                    tag="z_pad")
                    nc.gpsimd.memset(zt[:, 0:65], 0.0)
                    nc.gpsimd.memset(zt[:, ZP - 65:ZP], 0.0)
                    z_pad.append(zt)
                for nt in range(NT):
                    for mc in range(2):
                        zp = ps2.tile([128, 512], F32, space="PSUM",
                                      name="z_ps", tag="z_ps")
                        for kc in range(2):
                            nc.tensor.matmul(
                                zp[:], at_ap(kc, mc),
                                h_new[kc][:, 512 * nt:512 * nt + 512],
                                start=(kc == 0), stop=(kc == 1))
                        nc.scalar.activation(
                            z_pad[mc][:, 65 + 512 * nt:65 + 512 * nt + 512],
                            zp[:], COPY, scale=1.0 / ASC)

                # ---- attention: per 128-pixel chunk
                v_sb = {}
                for k in range(NCHUNK + 1):
                    if k < NCHUNK:
                        # v[k] = (Wv h)^T via lhsT = h_new columns
                        vps = ps2.tile([128, 256], F32, space="PSUM",
                                       name="v_ps", tag="v_ps")
                        for kc in range(2):
                            nc.tensor.matmul(
                                vps[:], h_new[kc][:, 128 * k:128 * k + 128],
                                wvt_ap(kc), start=(kc == 0), stop=(kc == 1))
                        vt = vpool.tile([128, 256], BF16, name="v_sb",
                                        tag="v_sb")
                        nc.vector.tensor_copy(vt[:], vps[:])
                        v_sb[k] = vt
                    if k < 1:
                        continue
                    j = k - 1
                    # Gram G = h^T z over the 258-wide band
                    gps = ps3.tile([128, 258], F32, space="PSUM",
                                   name="g_ps", tag="g_ps")
                    for kc in range(2):
                        nc.tensor.matmul(
                            gps[:], h_new[kc][:, 128 * j:128 * j + 128],
                            z_pad[kc][:, 128 * j:128 * j + 258],
                            start=(kc == 0), stop=(kc == 1))
                    gsb = small.tile([128, 258], BF16, name="gsb", tag="gsb")
                    nc.scalar.activation(gsb[:], gps[:], COPY)
                    gd = gdram.tile([128, 258], BF16, space="DRAM",
                                    name="g_dram", tag="g_dram")
                    nc.sync.dma_start(gd[:], gsb[:])
                    # diagonal extraction: s[p, (dy,dx)] = G[p, p+64(dy+1)+dx+1]
                    sc = small.tile([128, 9], BF16, name="sc", tag="sc")
                    for a in range(3):
                        nc.sync.dma_start(
                            sc[:, 3 * a:3 * a + 3],
                            _cap(gd, gd.offset + 64 * a,
                                 [(259, 128), (1, 3)]))
                    # mask -> exp -> normalize(+mask numerator)
                    sm = small.tile([128, 9], F32, name="sm", tag="sm")
                    nc.vector.tensor_tensor(sm[:], sc[:], mask_ap, op=MULT)
                    ex = small.tile([128, 9], F32, name="ex", tag="ex")
                    nc.scalar.activation(ex[:], sm[:], EXP)
                    sume = small.tile([128, 1], F32, name="sume", tag="sume")
                    nc.vector.tensor_reduce(sume[:], ex[:],
                                            axis=mybir.AxisListType.X, op=ADD)
                    rec = small.tile([128, 1], F32, name="rec", tag="rec")
                    nc.vector.reciprocal(rec[:], sume[:])
                    wn = small.tile([128, 9], BF16, name="wn", tag="wn")
                    nc.vector.scalar_tensor_tensor(
                        out=wn[:], in0=ex[:], scalar=rec[:], in1=mask_ap,
                        op0=MULT, op1=MULT)
                    # scatter normalized weights into banded W' in DRAM
                    wp = wpdram.tile([384, 128], BF16, space="DRAM",
                                     name="wp_dram", tag="wp_dram")
                    nc.sync.dma_start(wp[:], wpz[:])  # zero background
                    for a in range(3):
                        nc.sync.dma_start(
                            _cap(wp, wp.offset + 8064 + 8192 * a,
                                 [(129, 128), (128, 3)]),
                            wn[:, 3 * a:3 * a + 3])
                    wl = []
                    for j3 in range(3):
                        wlt = wlp.tile([128, 128], BF16, name="wl", tag="wl")
                        nc.sync.dma_start(
                            wlt[:], wp[128 * j3:128 * j3 + 128, :])
                        wl.append(wlt)
                    # h^T (identity matmul) + W'^T v_band, one PSUM group
                    fp = ps2.tile([128, 256], F32, space="PSUM",
                                  name="fin_ps", tag="fin_ps")
                    for kc in range(2):
                        nc.tensor.matmul(
                            fp[:], h_new[kc][:, 128 * j:128 * j + 128],
                            i256_ap(kc), start=(kc == 0), stop=False)
                    for j3 in range(3):
                        kk = j - 1 + j3
                        vband = v_sb[kk][:] if 0 <= kk < NCHUNK else vzero[:]
                        nc.tensor.matmul(fp[:], wl[j3][:], vband,
                                         start=False, stop=(j3 == 2))
                    # delta chunk = psum - x, evicted straight to fp8
                    osb = small.tile([128, 256], F8E3, name="osb", tag="osb")
                    nc.vector.tensor_tensor(osb[:], fp[:], hh[:, j, :],
                                            op=SUB)
                    nc.sync.dma_start(
                        _cap(out_d, (img * HW + 128 * j) * 256,
                             [(256, 128), (1, 256)]),
                        osb[:])

            for img in [i % BS for i in range(BS * reps)]:
                run_image(img)

    nc.compile()
    return nc


_NC_CACHE = {}


def _get_program():
    if "nc" not in _NC_CACHE:
        _NC_CACHE["nc"] = _build_program()
    return _NC_CACHE["nc"]


def _host_prepare(w_perc, b_perc, w_up1, b_up1, w_up2, b_up2, w_qkv, b_qkv):
    w_perc = np.asarray(w_perc, np.float32)
    b_perc = np.asarray(b_perc, np.float32)
    w_up1 = np.asarray(w_up1, np.float32)
    b_up1 = np.asarray(b_up1, np.float32)
    w_up2 = np.asarray(w_up2, np.float32)
    b_up2 = np.asarray(b_up2, np.float32)
    w_qkv = np.asarray(w_qkv, np.float32)
    b_qkv = np.asarray(b_qkv, np.float32)
    assert np.allclose(b_qkv, 0.0), "kernel assumes zero qkv bias (A-trick)"

    wp = w_perc[:, 0]                       # [3C, 3, 3]
    W1 = w_up1[:, :, 0, 0]                  # [2C, 3C]
    bh = b_up1 + W1 @ b_perc                # [2C]
    W2 = w_up2[:, :, 0, 0]                  # [C, 2C]
    Wq, Wk, Wv = w_qkv[:C], w_qkv[C:C2], w_qkv[C2:]
    A = (Wq.T @ Wk) / math.sqrt(C)          # [C, C]

    # w1t[t,cc,mc][g,d] = W1SC * W1[128mc+d, 3*(128cc+g)+t]
    W1r = W1.reshape(C2, C, 3)              # [d, g, t]
    w1t = np.empty((3, 2, 4, 128, 128), np.float32)
    for t in range(3):
        for cc in range(2):
            for mc in range(4):
                w1t[t, cc, mc] = (W1SC *
                                  W1r[mc * 128:(mc + 1) * 128,
                                      cc * 128:(cc + 1) * 128, t].T)
    # taps: wps[cc][g, t*9+tau] = wp[128cc+g, t, tau] / W1SC
    wpr = wp.reshape(C, 3, 9) / W1SC        # [g, t, tau]

    # wpk8 = [at x ASC | i256] e4m3
    wpk8 = np.empty((128, 1024), np.float32)
    for kc in range(2):
        for mc in range(2):
            wpk8[:, (kc * 2 + mc) * 128:(kc * 2 + mc) * 128 + 128] = \
                ASC * A[mc * 128:(mc + 1) * 128, kc * 128:(kc + 1) * 128].T
    eye = np.eye(256, dtype=np.float32).reshape(2, 128, 256)
    for kc in range(2):
        wpk8[:, 512 + kc * 256:512 + kc * 256 + 256] = eye[kc]

    # wpkb = [w2t | wvt] bf16
    wpkb = np.empty((128, 1536), np.float32)
    for kc in range(4):
        for mc in range(2):
            wpkb[:, (kc * 2 + mc) * 128:(kc * 2 + mc) * 128 + 128] = \
                W2[mc * 128:(mc + 1) * 128, kc * 128:(kc + 1) * 128].T
    WvT = Wv.T.reshape(2, 128, 256)
    for kc in range(2):
        wpkb[:, 1024 + kc * 256:1024 + kc * 256 + 256] = WvT[kc]

    # bk = [bh | b2 | mask | taps]
    bk = np.zeros((128, 69), np.float32)
    bk[:, 0:4] = bh.reshape(4, 128).T
    bk[:, 4:6] = b_up2.reshape(2, 128).T
    maskt = np.ones((128, 9), np.float32)
    for p in range(128):
        xx = p % 64
        for dy in (-1, 0, 1):
            for dx in (-1, 0, 1):
                if (xx == 0 and dx == -1) or (xx == 63 and dx == 1):
                    maskt[p, (dy + 1) * 3 + (dx + 1)] = 0.0
    bk[:, 6:15] = maskt
    for cc in range(2):
        bk[:, 15 + cc * 27:15 + cc * 27 + 27] = \
            wpr[cc * 128:(cc + 1) * 128].reshape(128, 27)

    return dict(w1t=w1t.astype(NP_E4), wpk8=wpk8.astype(NP_E4),
                wpkb=wpkb.astype(NP_BF), bk=bk)


def _pack_h(h):
    """h [B,H,W,C] f32 -> per-core fp8 NHWC chunk views (one global cast)."""
    h8 = np.ascontiguousarray(h).astype(NP_E3)
    return [h8[core * BS:(core + 1) * BS].reshape(BS, 32, 128, 256)
            for core in range(NCORES)]


def kernel(h, w_perc, b_perc, w_up1, b_up1, w_up2, b_up2, w_qkv, b_qkv):
    h = np.asarray(h, np.float32)
    consts = _host_prepare(w_perc, b_perc, w_up1, b_up1, w_up2, b_up2,
                           w_qkv, b_qkv)
    nc = _get_program()

    in_maps = []
    for hraw in _pack_h(h):
        m = {"hraw": hraw}
        m.update(consts)
        in_maps.append(m)

    res = run_bass_kernel_spmd(nc, in_maps, core_ids=list(range(NCORES)),
                               trace=False)
    delta = np.concatenate([res.results[i]["out"] for i in range(NCORES)], 0)
    return h + delta.astype(np.float32)
